# revision 45
# baseline (speedup 1.0000x reference)
"""Multi-head causal self-attention on 8 Trainium2 NeuronCores.

Problem: B=8, T=1024, D=1024, H=16 heads, DH=64.
    q,k,v = einsum('btd,hdk->bhtk', x, W{q,k,v})
    scores = q @ k.T / sqrt(DH), causal mask, softmax
    out = (softmax @ v) reshaped -> [B,T,H*DH] @ Wo + bo

Sharding: batch-parallel, one batch element per core (B == n_cores == 8).
No collectives; weights replicated to every core.

v2 (bf16): all matmul operands are bfloat16 (rel err ~4e-3 vs the 2e-2
gate).  On this hardware a matmul instruction costs ~free_size cycles
regardless of dtype, but the implicit per-matmul LDWEIGHTS is ~4x cheaper
for 2-byte weights (~70ns vs ~285ns for a 128-row stationary), DMA bytes
halve, and fp32-mode power throttling (30% of the baseline ran at a 50%
util cap) is avoided.  walrus ignores InstMatmult.ldweights=False and
--enable-ldw-opt crashes codegen, so every matmul self-loads; the layout
below minimizes ldw rows instead.

Per-core dataflow:
  xt [d,t] host-transposed, d on partitions.
  V-pass (xt stationary): V[t, h*dh] for ALL heads in [128t, 1024] psum
    tiles, 2 x 512-free matmuls per (tt,kd) ldw -> v1[t, h, tt, dh+1]
    with a ones column (row dh of the AV psum then yields the softmax
    denominator for free).
  QK-pass (weight stationary): QT/KT come out directly as [128(2 heads
    pair-packed on dh), t] -- no transposes.
  Attention per pair, staggered one s-tile: ST[s,q] = KT_j.T @ QT with
    exact causal trim (q >= j*128 only), exp on ACT (no max-subtraction;
    scores are O(6)), diagonal-block mask via gpsimd affine_select, then
    AV accumulates (V|1).T @ exp(ST) into [65, 512] psums per (head,
    q-chunk).  QK(p+1) projections are emitted between attention pairs
    so the PE always has ~2x more queued work than ACT needs to keep up.
  Normalization is deferred: unnormalized AV + denominator rows park in
    SBUF; phase 3 runs batched ACT reciprocals (ONE table switch -- Exp
    and Reciprocal never share an ACT table), PE ones-matmul broadcasts,
    DVE column-scale, ordered c0-chunks-first so the Wo projection of
    q-tiles 0..3 overlaps the c1 normalizations.
  Wo: final[q,d] = sum_pp OT[:,pp,q].T @ Wo[pp-rows, d] + bo, f32 out.

This walrus build allows ONE sync-wait per instruction, so a
post-scheduling pass hoists extra waits onto inserted PE no-ops.
"""

import sys

for _p in ("/opt/trn_rl_repo", "/root/.axon_site/_ro/trn_rl_repo"):
    if _p not in sys.path:
        sys.path.insert(0, _p)

import numpy as np

import concourse.bass as bass
import concourse.mybir as mybir
import concourse.tile as tile

f32 = mybir.dt.float32
bf16 = mybir.dt.bfloat16

B, T, D, H, DH = 8, 1024, 1024, 16, 64
NP = 128            # partitions
NC = 512            # matmul free-dim chunk (moving-operand max)
KT_ = D // NP       # 8 contraction tiles over d
NT = T // NP        # 8 tiles over t (s and q tiles)
NCH = T // NC       # 2 free-dim chunks over q
NPAIR = H // 2      # 8 head pairs (QT/KT pack 2 heads on partitions)


def build_nc(split_waits=True):
    nc = bass.Bass(trn_type="TRN2")
    xt = nc.dram_tensor("xt", [D, T], bf16, kind="ExternalInput")
    wq = nc.dram_tensor("wq", [D, H * DH], bf16, kind="ExternalInput")
    wk = nc.dram_tensor("wk", [D, H * DH], bf16, kind="ExternalInput")
    wv = nc.dram_tensor("wv", [D, H * DH], bf16, kind="ExternalInput")
    wo = nc.dram_tensor("wo", [H * DH, D], bf16, kind="ExternalInput")
    bo = nc.dram_tensor("bo", [1, D], bf16, kind="ExternalInput")
    out = nc.dram_tensor("out", [T, D], f32, kind="ExternalOutput")

    rec_dram = nc.dram_tensor("rec_scratch", [H, NCH * NC], bf16, kind="Internal")

    with tile.TileContext(nc) as tc:
        _mha(tc, nc, xt, wq, wk, wv, wo, bo, out, rec_dram)

    if split_waits:
        _split_waits(nc)
    return nc


def _mha(tc, nc, xt, wq, wk, wv, wo, bo, out, rec_dram):
    import contextlib

    ctx = contextlib.ExitStack()
    singles = ctx.enter_context(tc.tile_pool(name="singles", bufs=1))
    bigpool = ctx.enter_context(tc.tile_pool(name="bigpool", bufs=1))
    wpool = ctx.enter_context(tc.tile_pool(name="wpool", bufs=1))
    pexpool = ctx.enter_context(tc.tile_pool(name="pexpool", bufs=2))
    bcpool = ctx.enter_context(tc.tile_pool(name="bcpool", bufs=2))
    fpool = ctx.enter_context(tc.tile_pool(name="fpool", bufs=4))

    def act_recip(out_ap, in_ap):
        """ACT-engine reciprocal via raw InstActivation (nc.scalar.activation
        refuses Reciprocal; ~1.5e-6 rel err on our denominator range)."""
        ins = [nc.scalar.lower_ap(in_ap)]
        for arg in (0.0, 1.0, 0.0):                     # bias, scale, alpha
            ins.append(mybir.ImmediateValue(dtype=f32, value=arg))
        nc.scalar.add_instruction(mybir.InstActivation(
            name=nc.get_next_instruction_name(),
            func=mybir.ActivationFunctionType.Reciprocal,
            ins=ins,
            outs=[nc.scalar.lower_ap(out_ap)],
        ))

    with ctx:
        # --- resident tiles --------------------------------------------------
        onesf = singles.tile([NP, 1], f32)
        nc.vector.memset(onesf, 1.0)
        bo_bc = singles.tile([NP, D], bf16)
        nc.sync.dma_start(out=bo_bc, in_=bo[0:1, :].to_broadcast((NP, D)))

        # Inputs arrive as SEPARATE tiles in first-use order (the tile
        # framework tracks DMA deps per tile, so one big tile would make the
        # first matmul wait for every chunk): x^T per-kd [128, t], Wq/Wk
        # per-pair column slices [128, kd, 128], Wv per-kd.
        xt_k = [wpool.tile([NP, T], bf16, tag=f"xt{kd}", name=f"xt{kd}")
                for kd in range(KT_)]
        wq_p = [wpool.tile([NP, KT_, NP], bf16, tag=f"wq{p}", name=f"wq{p}")
                for p in range(NPAIR)]
        wk_p = [wpool.tile([NP, KT_, NP], bf16, tag=f"wk{p}", name=f"wk{p}")
                for p in range(NPAIR)]
        wv_sb = wpool.tile([NP, KT_, H * DH], bf16, tag="wv")

        def dma_w_slice(dst, w, pair):
            nc.sync.dma_start(
                out=dst,
                in_=w[:, pair * NP:(pair + 1) * NP].rearrange(
                    "(kt p) c -> p kt c", p=NP))

        nc.sync.dma_start(out=xt_k[0], in_=xt[0:NP, :])
        dma_w_slice(wq_p[0], wq, 0)
        dma_w_slice(wk_p[0], wk, 0)
        for kd in range(1, KT_):
            nc.sync.dma_start(out=xt_k[kd], in_=xt[kd * NP:(kd + 1) * NP, :])
        dma_w_slice(wq_p[1], wq, 1)
        dma_w_slice(wk_p[1], wk, 1)
        for kd in range(KT_):
            nc.sync.dma_start(out=wv_sb[:, kd, :], in_=wv[kd * NP:(kd + 1) * NP, :])
        for p in range(2, NPAIR):
            dma_w_slice(wq_p[p], wq, p)
            dma_w_slice(wk_p[p], wk, p)

        qt_sb = singles.tile([NP, NPAIR, T], bf16, name="qt_sb")
        kt_sb = singles.tile([NP, NPAIR, T], bf16, name="kt_sb")
        v1_sb = singles.tile([NP, H, NT, DH + 1], bf16, name="v1_sb")
        ot_sb = singles.tile([NP, NPAIR, T], bf16, name="ot_sb")
        # unnormalized AV + denominator row: [65, h, c, 512]
        avsb = singles.tile([DH + 1, H, NCH, NC], bf16, name="avsb")

        nc.vector.tensor_copy(
            out=v1_sb[:, :, :, DH:DH + 1],
            in_=onesf.to_broadcast((NP, H, NT, 1)))

        psctx = contextlib.ExitStack()
        ps_qk = psctx.enter_context(tc.tile_pool(name="ps_qk", bufs=2, space="PSUM"))
        ps_st = psctx.enter_context(tc.tile_pool(name="ps_st", bufs=2, space="PSUM"))

        def emit_qk(pair):
            for w_p, dst in ((wq_p, qt_sb), (wk_p, kt_sb)):
                for c in range(NCH):
                    ps = ps_qk.tile([NP, NC], f32, tag="qk", name="qk_ps")
                    for kd in range(KT_):
                        nc.tensor.matmul(
                            out=ps,
                            lhsT=w_p[pair][:, kd, :],
                            rhs=xt_k[kd][:, c * NC:(c + 1) * NC],
                            start=(kd == 0), stop=(kd == KT_ - 1),
                        )
                    nc.vector.tensor_copy(
                        out=dst[:, pair, c * NC:(c + 1) * NC], in_=ps)

        av_tiles = {}

        def emit_att(pair):
            # av psums [65, 512] per (hh, c); reused ring=1 across pairs
            for hh in range(2):
                for c in range(NCH):
                    av_tiles[(hh, c)] = ps_av.tile(
                        [DH + 1, NC], f32, tag=f"av{hh}{c}", name="av_ps")

            def emit_st(j):
                a0 = j * NP
                outp = []
                for hh in range(2):
                    hp = hh * DH
                    px = pexpool.tile([NP, T], bf16, tag=f"px{hh}", name="px")
                    for c in range(NCH):
                        lo = max(a0, c * NC)
                        if lo >= (c + 1) * NC:
                            continue
                        st = ps_st.tile([NP, NC], f32, tag="st", name="st_ps")
                        nc.tensor.matmul(
                            out=st[:, lo - c * NC:NC],
                            lhsT=kt_sb[hp:hp + DH, pair, a0:a0 + NP],
                            rhs=qt_sb[hp:hp + DH, pair, lo:(c + 1) * NC],
                            start=True, stop=True,
                        )
                        nc.scalar.activation(
                            out=px[:, lo:(c + 1) * NC],
                            in_=st[:, lo - c * NC:NC],
                            func=mybir.ActivationFunctionType.Exp)
                    # causal mask on the diagonal 128-col block only
                    nc.gpsimd.affine_select(
                        out=px[:, a0:a0 + NP], in_=px[:, a0:a0 + NP],
                        pattern=[[1, NP]],
                        compare_op=mybir.AluOpType.is_ge,
                        fill=0.0, base=0, channel_multiplier=-1,
                    )
                    outp.append(px)
                return outp

            def emit_av(j, pexp_j):
                a0 = j * NP
                for hh in range(2):
                    h = 2 * pair + hh
                    for c in range(NCH):
                        lo = max(a0, c * NC)
                        if lo >= (c + 1) * NC:
                            continue
                        nc.tensor.matmul(
                            out=av_tiles[(hh, c)][:, lo - c * NC:NC],
                            lhsT=v1_sb[:, h, j, :],
                            rhs=pexp_j[hh][:, lo:(c + 1) * NC],
                            start=(j == 0),
                            stop=(j == (NT - 1 if c else NT // NCH - 1)),
                            skip_group_check=True,
                        )

            prev = None
            for j in range(NT):
                cur = (j, emit_st(j))
                if prev is not None:
                    emit_av(*prev)
                prev = cur
            emit_av(*prev)

            for hh in range(2):
                h = 2 * pair + hh
                for c in range(NCH):
                    nc.vector.tensor_copy(
                        out=avsb[:, h, c, :], in_=av_tiles[(hh, c)])
            # scatter this pair's denominator rows to partitions 2p..2p+1
            # of the 16-lane recip staging tile (overlaps attention)
            nc.sync.dma_start(
                out=den_sp[2 * pair:2 * pair + 2, 0:NCH, 0:NC],
                in_=avsb[DH:DH + 1, 2 * pair:2 * pair + 2, :, :])

        # QK for the first two pairs, then the V-pass (xt stationary, all
        # heads), then attention with QK kept two pairs ahead
        emit_qk(0)
        emit_qk(1)
        with tc.tile_pool(name="ps_v", bufs=2, space="PSUM") as ps_v:
            for tt in range(NT):
                psv = ps_v.tile([NP, H, DH], f32, tag="v", name="psv")
                for kd in range(KT_):
                    for half in range(2):
                        nc.tensor.matmul(
                            out=psv[:, half * 8:(half + 1) * 8, :],
                            lhsT=xt_k[kd][:, tt * NP:(tt + 1) * NP],
                            rhs=wv_sb[:, kd, half * NC:(half + 1) * NC],
                            start=(kd == 0), stop=(kd == KT_ - 1),
                        )
                nc.vector.tensor_copy(out=v1_sb[:, :, tt, 0:DH], in_=psv)
        # dead wv slot reused for the denominator spread (one head per
        # partition, so the ACT reciprocal runs 16 lanes wide)
        den_sp = wpool.tile([NP, KT_, H * DH], bf16, tag="wv", name="den_sp")
        ps_av = psctx.enter_context(tc.tile_pool(name="ps_av", bufs=1, space="PSUM"))
        for pair in range(NPAIR):
            if pair + 2 < NPAIR:
                emit_qk(pair + 2)
            emit_att(pair)

        # Wo DMA into x^T's slot (x^T fully consumed by the QK pass)
        wo_sb = bigpool.tile([NP, NPAIR, D], bf16, tag="big", name="wo_sb")
        nc.sync.dma_start(out=wo_sb, in_=wo.rearrange("(kt p) d -> p kt d", p=NP))

        # Denominator rows (partition 64 of avsb) scatter to partitions
        # 0..15 via one SBUF->SBUF DMA, then ONE 16-lane in-place ACT
        # reciprocal.  Its input spans every head, so its deps include pair
        # 7 and the scheduler cannot hoist it into the attention stream
        # (that would thrash the exp<->recip ACT tables).  1/den then
        # roundtrips through a DRAM scratch so the broadcast can use a
        # stride-0-partition DMA read (SBUF sources reject that).
        den_c = den_sp[0:H, 0:NCH, 0:NC]
        act_recip(den_c, den_c)
        nc.sync.dma_start(
            out=rec_dram[:, :].rearrange("h (c n) -> h c n", c=NCH), in_=den_c)

        psctx.close()  # release ps_qk/ps_st/ps_av banks
        with tc.tile_pool(name="ps_wo", bufs=2, space="PSUM") as ps_wo:

            # --- deferred softmax normalization ------------------------------
            def emit_norm(c):
                for h in range(H):
                    bc = bcpool.tile([DH, NC], bf16, tag="bc", name="bc_sb")
                    eng = nc.sync if h % 2 == 0 else nc.gpsimd
                    eng.dma_start(
                        out=bc,
                        in_=rec_dram[h:h + 1, c * NC:(c + 1) * NC]
                        .to_broadcast((DH, NC)))
                    nc.vector.tensor_mul(
                        out=ot_sb[(h % 2) * DH:(h % 2 + 1) * DH,
                                  h // 2, c * NC:(c + 1) * NC],
                        in0=avsb[0:DH, h, c, :],
                        in1=bc,
                    )

            def emit_wo(qi):
                for dc in range(NCH):
                    ps = ps_wo.tile([NP, NC], f32, tag=f"wo{dc}", name="wo_ps")
                    for pp in range(NPAIR):
                        nc.tensor.matmul(
                            out=ps,
                            lhsT=ot_sb[:, pp, qi * NP:(qi + 1) * NP],
                            rhs=wo_sb[:, pp, dc * NC:(dc + 1) * NC],
                            start=(pp == 0), stop=(pp == NPAIR - 1),
                        )
                    f_sb = fpool.tile([NP, NC], f32, tag="f", name="f_sb")
                    nc.vector.tensor_add(
                        out=f_sb, in0=ps, in1=bo_bc[:, dc * NC:(dc + 1) * NC])
                    nc.sync.dma_start(
                        out=out[qi * NP:(qi + 1) * NP, dc * NC:(dc + 1) * NC],
                        in_=f_sb)

            emit_norm(0)
            emit_norm(1)
            for qi in range(NT):
                emit_wo(qi)


def _split_waits(nc, max_waits=1):
    """Walrus on this target allows one sync-wait per instruction; hoist
    extras onto no-ops inserted just before the offending instruction."""
    for f in nc.m.functions:
        for b in f.blocks:
            insts = b.instructions
            new = []
            changed = False
            for inst in insts:
                si = inst.sync_info
                if si is not None and len(si.on_wait) > max_waits:
                    waits = list(si.on_wait)
                    extra, keep = waits[:-max_waits], waits[-max_waits:]
                    for j, w in enumerate(extra):
                        new.append(mybir.InstNoOp(
                            name=f"{inst.name}-wnop{j}",
                            sync_info=mybir.SyncInfo(on_wait=[w], on_update=[]),
                            engine=inst.engine,
                            bass_nofuse=True,
                        ))
                    inst.sync_info = mybir.SyncInfo(
                        on_wait=keep, on_update=list(si.on_update))
                    changed = True
                new.append(inst)
            if changed:
                b.instructions = new


def make_in_maps(x, Wq, Wk, Wv, Wo, bo):
    import ml_dtypes
    nbf = ml_dtypes.bfloat16
    scale = np.float32(DH) ** np.float32(-0.5)
    # [H, D, DH] -> [D, H*DH]; fold the 1/sqrt(DH) score scale into Wq
    wq_m = np.ascontiguousarray(
        np.asarray(Wq).transpose(1, 0, 2).reshape(D, H * DH) * scale).astype(nbf)
    wk_m = np.ascontiguousarray(
        np.asarray(Wk).transpose(1, 0, 2).reshape(D, H * DH)).astype(nbf)
    wv_m = np.ascontiguousarray(
        np.asarray(Wv).transpose(1, 0, 2).reshape(D, H * DH)).astype(nbf)
    wo_m = np.ascontiguousarray(np.asarray(Wo)).astype(nbf)
    bo_m = np.ascontiguousarray(np.asarray(bo).reshape(1, D)).astype(nbf)
    return [
        {
            "xt": np.ascontiguousarray(np.asarray(x[b]).T).astype(nbf),
            "wq": wq_m, "wk": wk_m, "wv": wv_m, "wo": wo_m, "bo": bo_m,
        }
        for b in range(B)
    ]


_NC_CACHE = []


def kernel(x, Wq, Wk, Wv, Wo, bo):
    from concourse.bass_utils import run_bass_kernel_spmd

    x = np.asarray(x)
    if not _NC_CACHE:
        _NC_CACHE.append(build_nc())
    nc = _NC_CACHE[0]
    in_maps = make_in_maps(x, np.asarray(Wq), np.asarray(Wk), np.asarray(Wv),
                           np.asarray(Wo), np.asarray(bo))
    res = run_bass_kernel_spmd(nc, in_maps, core_ids=list(range(B)))
    return np.stack([res.results[b]["out"] for b in range(B)]).astype(np.float32)


# revision 46
# speedup vs baseline: 1.0123x; 1.0123x over previous
"""Multi-head causal self-attention on 8 Trainium2 NeuronCores.

Problem: B=8, T=1024, D=1024, H=16 heads, DH=64.
    q,k,v = einsum('btd,hdk->bhtk', x, W{q,k,v})
    scores = q @ k.T / sqrt(DH), causal mask, softmax
    out = (softmax @ v) reshaped -> [B,T,H*DH] @ Wo + bo

Sharding: batch-parallel, one batch element per core (B == n_cores == 8).
No collectives; weights replicated to every core.

v2 (bf16): all matmul operands are bfloat16 (rel err ~4e-3 vs the 2e-2
gate).  On this hardware a matmul instruction costs ~free_size cycles
regardless of dtype, but the implicit per-matmul LDWEIGHTS is ~4x cheaper
for 2-byte weights (~70ns vs ~285ns for a 128-row stationary), DMA bytes
halve, and fp32-mode power throttling (30% of the baseline ran at a 50%
util cap) is avoided.  walrus ignores InstMatmult.ldweights=False and
--enable-ldw-opt crashes codegen, so every matmul self-loads; the layout
below minimizes ldw rows instead.

Per-core dataflow:
  xt [d,t] host-transposed, d on partitions.
  V-pass (xt stationary): V[t, h*dh] for ALL heads in [128t, 1024] psum
    tiles, 2 x 512-free matmuls per (tt,kd) ldw -> v1[t, h, tt, dh+1]
    with a ones column (row dh of the AV psum then yields the softmax
    denominator for free).
  QK-pass (weight stationary): QT/KT come out directly as [128(2 heads
    pair-packed on dh), t] -- no transposes.
  Attention per pair, staggered one s-tile: ST[s,q] = KT_j.T @ QT with
    exact causal trim (q >= j*128 only), exp on ACT (no max-subtraction;
    scores are O(6)), diagonal-block mask via gpsimd affine_select, then
    AV accumulates (V|1).T @ exp(ST) into [65, 512] psums per (head,
    q-chunk).  QK(p+1) projections are emitted between attention pairs
    so the PE always has ~2x more queued work than ACT needs to keep up.
  Normalization is deferred: unnormalized AV + denominator rows park in
    SBUF; phase 3 runs batched ACT reciprocals (ONE table switch -- Exp
    and Reciprocal never share an ACT table), PE ones-matmul broadcasts,
    DVE column-scale, ordered c0-chunks-first so the Wo projection of
    q-tiles 0..3 overlaps the c1 normalizations.
  Wo: final[q,d] = sum_pp OT[:,pp,q].T @ Wo[pp-rows, d] + bo, f32 out.

This walrus build allows ONE sync-wait per instruction, so a
post-scheduling pass hoists extra waits onto inserted PE no-ops.
"""

import sys

for _p in ("/opt/trn_rl_repo", "/root/.axon_site/_ro/trn_rl_repo"):
    if _p not in sys.path:
        sys.path.insert(0, _p)

import numpy as np

import concourse.bass as bass
import concourse.mybir as mybir
import concourse.tile as tile

f32 = mybir.dt.float32
bf16 = mybir.dt.bfloat16

B, T, D, H, DH = 8, 1024, 1024, 16, 64
NP = 128            # partitions
NC = 512            # matmul free-dim chunk (moving-operand max)
KT_ = D // NP       # 8 contraction tiles over d
NT = T // NP        # 8 tiles over t (s and q tiles)
NCH = T // NC       # 2 free-dim chunks over q
NPAIR = H // 2      # 8 head pairs (QT/KT pack 2 heads on partitions)


def build_nc(split_waits=True):
    nc = bass.Bass(trn_type="TRN2")
    xt = nc.dram_tensor("xt", [D, T], bf16, kind="ExternalInput")
    wq = nc.dram_tensor("wq", [D, H * DH], bf16, kind="ExternalInput")
    wk = nc.dram_tensor("wk", [D, H * DH], bf16, kind="ExternalInput")
    wv = nc.dram_tensor("wv", [D, H * DH], bf16, kind="ExternalInput")
    wo = nc.dram_tensor("wo", [H * DH, D], bf16, kind="ExternalInput")
    bo = nc.dram_tensor("bo", [1, D], bf16, kind="ExternalInput")
    out = nc.dram_tensor("out", [T, D], f32, kind="ExternalOutput")

    rec_dram = nc.dram_tensor("rec_scratch", [H, NCH * NC], bf16, kind="Internal")

    with tile.TileContext(nc) as tc:
        _mha(tc, nc, xt, wq, wk, wv, wo, bo, out, rec_dram)

    if split_waits:
        _split_waits(nc)
    return nc


def _mha(tc, nc, xt, wq, wk, wv, wo, bo, out, rec_dram):
    import contextlib

    ctx = contextlib.ExitStack()
    singles = ctx.enter_context(tc.tile_pool(name="singles", bufs=1))
    bigpool = ctx.enter_context(tc.tile_pool(name="bigpool", bufs=1))
    wpool = ctx.enter_context(tc.tile_pool(name="wpool", bufs=1))
    pexpool = ctx.enter_context(tc.tile_pool(name="pexpool", bufs=2))
    bcpool = ctx.enter_context(tc.tile_pool(name="bcpool", bufs=2))
    fpool = ctx.enter_context(tc.tile_pool(name="fpool", bufs=2))

    def act_recip(out_ap, in_ap):
        """ACT-engine reciprocal via raw InstActivation (nc.scalar.activation
        refuses Reciprocal; ~1.5e-6 rel err on our denominator range)."""
        ins = [nc.scalar.lower_ap(in_ap)]
        for arg in (0.0, 1.0, 0.0):                     # bias, scale, alpha
            ins.append(mybir.ImmediateValue(dtype=f32, value=arg))
        nc.scalar.add_instruction(mybir.InstActivation(
            name=nc.get_next_instruction_name(),
            func=mybir.ActivationFunctionType.Reciprocal,
            ins=ins,
            outs=[nc.scalar.lower_ap(out_ap)],
        ))

    with ctx:
        # --- resident tiles --------------------------------------------------
        onesf = singles.tile([NP, 1], f32)
        nc.vector.memset(onesf, 1.0)
        bo_bc = singles.tile([NP, D], bf16)
        nc.sync.dma_start(out=bo_bc, in_=bo[0:1, :].to_broadcast((NP, D)))

        # Inputs arrive as SEPARATE tiles in first-use order (the tile
        # framework tracks DMA deps per tile, so one big tile would make the
        # first matmul wait for every chunk): x^T per-kd [128, t], Wq/Wk
        # per-pair column slices [128, kd, 128], Wv per-kd.
        xt_k = [wpool.tile([NP, T], bf16, tag=f"xt{kd}", name=f"xt{kd}")
                for kd in range(KT_)]
        wq_p = [wpool.tile([NP, KT_, NP], bf16, tag=f"wq{p}", name=f"wq{p}")
                for p in range(NPAIR)]
        wk_p = [wpool.tile([NP, KT_, NP], bf16, tag=f"wk{p}", name=f"wk{p}")
                for p in range(NPAIR)]
        wv_sb = wpool.tile([NP, KT_, H * DH], bf16, tag="wv")

        def dma_w_slice(dst, w, pair):
            nc.sync.dma_start(
                out=dst,
                in_=w[:, pair * NP:(pair + 1) * NP].rearrange(
                    "(kt p) c -> p kt c", p=NP))

        nc.sync.dma_start(out=xt_k[0], in_=xt[0:NP, :])
        dma_w_slice(wq_p[0], wq, 0)
        dma_w_slice(wk_p[0], wk, 0)
        for kd in range(1, KT_):
            nc.sync.dma_start(out=xt_k[kd], in_=xt[kd * NP:(kd + 1) * NP, :])
        dma_w_slice(wq_p[1], wq, 1)
        dma_w_slice(wk_p[1], wk, 1)
        for kd in range(KT_):
            nc.sync.dma_start(out=wv_sb[:, kd, :], in_=wv[kd * NP:(kd + 1) * NP, :])
        for p in range(2, NPAIR):
            dma_w_slice(wq_p[p], wq, p)
            dma_w_slice(wk_p[p], wk, p)

        qt_sb = singles.tile([NP, NPAIR, T], bf16, name="qt_sb")
        kt_sb = singles.tile([NP, NPAIR, T], bf16, name="kt_sb")
        v1_sb = singles.tile([NP, H, NT, DH + 1], bf16, name="v1_sb")
        ot_sb = singles.tile([NP, NPAIR, T], bf16, name="ot_sb")
        # unnormalized AV + denominator row: [65, h, c, 512]
        avsb = singles.tile([DH + 1, H, NCH, NC], bf16, name="avsb")

        nc.vector.tensor_copy(
            out=v1_sb[:, :, :, DH:DH + 1],
            in_=onesf.to_broadcast((NP, H, NT, 1)))

        psctx = contextlib.ExitStack()
        ps_qk = psctx.enter_context(tc.tile_pool(name="ps_qk", bufs=2, space="PSUM"))
        ps_st = psctx.enter_context(tc.tile_pool(name="ps_st", bufs=2, space="PSUM"))

        def emit_qk(pair):
            for w_p, dst in ((wq_p, qt_sb), (wk_p, kt_sb)):
                for c in range(NCH):
                    ps = ps_qk.tile([NP, NC], f32, tag="qk", name="qk_ps")
                    for kd in range(KT_):
                        nc.tensor.matmul(
                            out=ps,
                            lhsT=w_p[pair][:, kd, :],
                            rhs=xt_k[kd][:, c * NC:(c + 1) * NC],
                            start=(kd == 0), stop=(kd == KT_ - 1),
                        )
                    nc.vector.tensor_copy(
                        out=dst[:, pair, c * NC:(c + 1) * NC], in_=ps)

        av_tiles = {}

        def emit_att(pair):
            # av psums [65, 512] per (hh, c); reused ring=1 across pairs
            for hh in range(2):
                for c in range(NCH):
                    av_tiles[(hh, c)] = ps_av.tile(
                        [DH + 1, NC], f32, tag=f"av{hh}{c}", name="av_ps")

            def emit_st(j):
                a0 = j * NP
                outp = []
                for hh in range(2):
                    hp = hh * DH
                    px = pexpool.tile([NP, T], bf16, tag=f"px{hh}", name="px")
                    for c in range(NCH):
                        lo = max(a0, c * NC)
                        if lo >= (c + 1) * NC:
                            continue
                        st = ps_st.tile([NP, NC], f32, tag="st", name="st_ps")
                        nc.tensor.matmul(
                            out=st[:, lo - c * NC:NC],
                            lhsT=kt_sb[hp:hp + DH, pair, a0:a0 + NP],
                            rhs=qt_sb[hp:hp + DH, pair, lo:(c + 1) * NC],
                            start=True, stop=True,
                        )
                        nc.scalar.activation(
                            out=px[:, lo:(c + 1) * NC],
                            in_=st[:, lo - c * NC:NC],
                            func=mybir.ActivationFunctionType.Exp)
                    # causal mask on the diagonal 128-col block only
                    nc.gpsimd.affine_select(
                        out=px[:, a0:a0 + NP], in_=px[:, a0:a0 + NP],
                        pattern=[[1, NP]],
                        compare_op=mybir.AluOpType.is_ge,
                        fill=0.0, base=0, channel_multiplier=-1,
                    )
                    outp.append(px)
                return outp

            def emit_av(j, pexp_j):
                a0 = j * NP
                for hh in range(2):
                    h = 2 * pair + hh
                    for c in range(NCH):
                        lo = max(a0, c * NC)
                        if lo >= (c + 1) * NC:
                            continue
                        nc.tensor.matmul(
                            out=av_tiles[(hh, c)][:, lo - c * NC:NC],
                            lhsT=v1_sb[:, h, j, :],
                            rhs=pexp_j[hh][:, lo:(c + 1) * NC],
                            start=(j == 0),
                            stop=(j == (NT - 1 if c else NT // NCH - 1)),
                            skip_group_check=True,
                        )

            prev = None
            for j in range(NT):
                cur = (j, emit_st(j))
                if prev is not None:
                    emit_av(*prev)
                prev = cur
            emit_av(*prev)

            for hh in range(2):
                h = 2 * pair + hh
                for c in range(NCH):
                    nc.vector.tensor_copy(
                        out=avsb[:, h, c, :], in_=av_tiles[(hh, c)])

        # QK for the first two pairs, then the V-pass (xt stationary, all
        # heads), then attention with QK kept two pairs ahead
        emit_qk(0)
        emit_qk(1)
        with tc.tile_pool(name="ps_v", bufs=2, space="PSUM") as ps_v:
            for tt in range(NT):
                psv = ps_v.tile([NP, H, DH], f32, tag="v", name="psv")
                for kd in range(KT_):
                    for half in range(2):
                        nc.tensor.matmul(
                            out=psv[:, half * 8:(half + 1) * 8, :],
                            lhsT=xt_k[kd][:, tt * NP:(tt + 1) * NP],
                            rhs=wv_sb[:, kd, half * NC:(half + 1) * NC],
                            start=(kd == 0), stop=(kd == KT_ - 1),
                        )
                nc.vector.tensor_copy(out=v1_sb[:, :, tt, 0:DH], in_=psv)
        # dead wv slot reused for the denominator spread (one head per
        # partition, so the ACT reciprocal runs 16 lanes wide)
        den_sp = wpool.tile([NP, KT_, H * DH], bf16, tag="wv", name="den_sp")
        ps_av = psctx.enter_context(tc.tile_pool(name="ps_av", bufs=1, space="PSUM"))
        for pair in range(NPAIR):
            if pair + 2 < NPAIR:
                emit_qk(pair + 2)
            emit_att(pair)

        # Wo DMA into x^T's slot (x^T fully consumed by the QK pass)
        wo_sb = bigpool.tile([NP, NPAIR, D], bf16, tag="big", name="wo_sb")
        nc.sync.dma_start(out=wo_sb, in_=wo.rearrange("(kt p) d -> p kt d", p=NP))

        # Denominator rows (partition 64 of avsb) scatter to partitions
        # 0..15 via one SBUF->SBUF DMA, then ONE 16-lane in-place ACT
        # reciprocal.  Its input spans every head, so its deps include pair
        # 7 and the scheduler cannot hoist it into the attention stream
        # (that would thrash the exp<->recip ACT tables).  1/den then
        # roundtrips through a DRAM scratch so the broadcast can use a
        # stride-0-partition DMA read (SBUF sources reject that).
        den_c = den_sp[0:H, 0:NCH, 0:NC]
        nc.sync.dma_start(out=den_c, in_=avsb[DH:DH + 1, :, :, :])
        act_recip(den_c, den_c)
        nc.sync.dma_start(
            out=rec_dram[:, :].rearrange("h (c n) -> h c n", c=NCH), in_=den_c)

        psctx.close()  # release ps_qk/ps_st/ps_av banks
        with tc.tile_pool(name="ps_wo", bufs=2, space="PSUM") as ps_wo:

            # --- deferred softmax normalization ------------------------------
            def emit_norm(c):
                for h in range(H):
                    bc = bcpool.tile([DH, NC], bf16, tag="bc", name="bc_sb")
                    nc.sync.dma_start(
                        out=bc,
                        in_=rec_dram[h:h + 1, c * NC:(c + 1) * NC]
                        .to_broadcast((DH, NC)))
                    nc.vector.tensor_mul(
                        out=ot_sb[(h % 2) * DH:(h % 2 + 1) * DH,
                                  h // 2, c * NC:(c + 1) * NC],
                        in0=avsb[0:DH, h, c, :],
                        in1=bc,
                    )

            def emit_wo(qi):
                f_sb = fpool.tile([NP, D], f32, name="f_sb")
                for dc in range(NCH):
                    ps = ps_wo.tile([NP, NC], f32, tag=f"wo{dc}", name="wo_ps")
                    for pp in range(NPAIR):
                        nc.tensor.matmul(
                            out=ps,
                            lhsT=ot_sb[:, pp, qi * NP:(qi + 1) * NP],
                            rhs=wo_sb[:, pp, dc * NC:(dc + 1) * NC],
                            start=(pp == 0), stop=(pp == NPAIR - 1),
                        )
                    nc.vector.tensor_add(
                        out=f_sb[:, dc * NC:(dc + 1) * NC],
                        in0=ps,
                        in1=bo_bc[:, dc * NC:(dc + 1) * NC],
                    )
                nc.sync.dma_start(out=out[qi * NP:(qi + 1) * NP, :], in_=f_sb)

            emit_norm(0)
            for qi in range(NT // 2):
                emit_wo(qi)
            emit_norm(1)
            for qi in range(NT // 2, NT):
                emit_wo(qi)


def _split_waits(nc, max_waits=1):
    """Walrus on this target allows one sync-wait per instruction; hoist
    extras onto no-ops inserted just before the offending instruction."""
    for f in nc.m.functions:
        for b in f.blocks:
            insts = b.instructions
            new = []
            changed = False
            for inst in insts:
                si = inst.sync_info
                if si is not None and len(si.on_wait) > max_waits:
                    waits = list(si.on_wait)
                    extra, keep = waits[:-max_waits], waits[-max_waits:]
                    for j, w in enumerate(extra):
                        new.append(mybir.InstNoOp(
                            name=f"{inst.name}-wnop{j}",
                            sync_info=mybir.SyncInfo(on_wait=[w], on_update=[]),
                            engine=inst.engine,
                            bass_nofuse=True,
                        ))
                    inst.sync_info = mybir.SyncInfo(
                        on_wait=keep, on_update=list(si.on_update))
                    changed = True
                new.append(inst)
            if changed:
                b.instructions = new


def make_in_maps(x, Wq, Wk, Wv, Wo, bo):
    import ml_dtypes
    nbf = ml_dtypes.bfloat16
    scale = np.float32(DH) ** np.float32(-0.5)
    # [H, D, DH] -> [D, H*DH]; fold the 1/sqrt(DH) score scale into Wq
    wq_m = np.ascontiguousarray(
        np.asarray(Wq).transpose(1, 0, 2).reshape(D, H * DH) * scale).astype(nbf)
    wk_m = np.ascontiguousarray(
        np.asarray(Wk).transpose(1, 0, 2).reshape(D, H * DH)).astype(nbf)
    wv_m = np.ascontiguousarray(
        np.asarray(Wv).transpose(1, 0, 2).reshape(D, H * DH)).astype(nbf)
    wo_m = np.ascontiguousarray(np.asarray(Wo)).astype(nbf)
    bo_m = np.ascontiguousarray(np.asarray(bo).reshape(1, D)).astype(nbf)
    return [
        {
            "xt": np.ascontiguousarray(np.asarray(x[b]).T).astype(nbf),
            "wq": wq_m, "wk": wk_m, "wv": wv_m, "wo": wo_m, "bo": bo_m,
        }
        for b in range(B)
    ]


_NC_CACHE = []


def kernel(x, Wq, Wk, Wv, Wo, bo):
    from concourse.bass_utils import run_bass_kernel_spmd

    x = np.asarray(x)
    if not _NC_CACHE:
        _NC_CACHE.append(build_nc())
    nc = _NC_CACHE[0]
    in_maps = make_in_maps(x, np.asarray(Wq), np.asarray(Wk), np.asarray(Wv),
                           np.asarray(Wo), np.asarray(bo))
    res = run_bass_kernel_spmd(nc, in_maps, core_ids=list(range(B)))
    return np.stack([res.results[b]["out"] for b in range(B)]).astype(np.float32)
